# revision 3
# baseline (speedup 1.0000x reference)
"""Trainium2 Bass kernel for nn_Join: out = concat(unary[idx1], unary[idx2], binary).

Data-parallel over edges on 8 cores; the unary table is replicated per core
(as 4 row-chunks of <=32768 rows) so gathers are local.

The old per-128-row indirect-DMA design was SWDGE-launch-bound: each
gpsimd indirect_dma_start costs ~1us of Pool-engine time regardless of
payload (994ns fixed + 0.34ns/descriptor), and 2x977 launches/core put
2.2ms of serialized Pool time against a ~0.9ms HBM roofline.

This version batches descriptor generation with dma_gather/dma_scatter_add
(thousands of rows per launch):
  - Edges are processed in segments of 8192. For each (segment, tensor,
    chunk) the host compacts the edge indices hitting that table chunk
    (stable order -> ascending output positions) into int16 index lists
    (table-relative for the gather, segment-relative output row for the
    scatter); -1 padding up to a static capacity.
  - dma_gather packs the hit rows into SBUF slots; dma_scatter_add places
    them at their true output rows (elem_step=320) on top of the
    pre-zeroed output buffer. Each output row is written exactly once, so
    the CCE ADD acts as a plain write.
  - Valid-count per launch is runtime data (differs per core under SPMD),
    so counts ride in a [1,128] int32 tensor, reg_load'ed into a Pool
    scalar register and passed as num_idxs_reg. One program serves all 8
    cores; capacities are static (mean + >6 sigma).
  - binary is host-pre-permuted to a [seg, p, n*64] layout so the load is
    contiguous; the store interleaves into columns 256:320 of each row.

Pool-engine time drops to ~256 launches x 1us + 500k descriptors x 0.34ns
~= 0.45ms/core, under the HBM roofline, and overlaps with the DMA streams.
"""

import numpy as np
from contextlib import ExitStack

import concourse.bass as bass
import concourse.bacc as bacc
import concourse.tile as tile
import concourse.mybir as mybir
from concourse.bass_utils import run_bass_kernel_spmd

N_CORES = 8
U_NODES, U_DIM = 100000, 128
B_DIM = 64
OUT_DIM = 2 * U_DIM + B_DIM  # 320
P = 128
CHUNK = 32768
NCHUNK = -(-U_NODES // CHUNK)  # 4
CHUNK_SIZES = [min(CHUNK, U_NODES - c * CHUNK) for c in range(NCHUNK)]
SEG = 8192
SEGCOLS = SEG // P  # 64

# static per-(segment, tensor, chunk) capacity: mean + ~6 sigma, 128-rounded
def _cap(chunk_rows: int) -> int:
    frac = chunk_rows / U_NODES
    mean = SEG * frac
    sig = (SEG * frac * (1.0 - frac)) ** 0.5
    return max(128, int(-(-(mean + 8 * sig) // 128)) * 128)

CAPS = [_cap(r) for r in CHUNK_SIZES]
CAPCOLS = [c // P for c in CAPS]
CAPI = [-(-c // 16) for c in CAPS]
IDX_COLS = 2 * sum(CAPI)  # per-segment idx plane width (t0 chunks, t1 chunks)
TCOLS = sum(CAPCOLS)  # gather-tile cols per tensor


def _wrap16(vals: np.ndarray, cap_cols: int) -> np.ndarray:
    """int16 [n] -> [128, cap_cols]: entry i at (i%16, i//16), replicated x8."""
    pad = np.full(cap_cols * 16, -1, np.int16)
    pad[: len(vals)] = vals
    return np.tile(pad.reshape(cap_cols, 16).T, (8, 1))


def _build_nc(nseg: int):
    ne_pad = nseg * SEG
    nc = bacc.Bacc(
        "TRN2",
        target_bir_lowering=False,
        debug=False,
        enable_asserts=False,
        num_devices=N_CORES,
    )
    chunks = [
        nc.dram_tensor(f"uc{c}", [CHUNK_SIZES[c], U_DIM], mybir.dt.float32,
                       kind="ExternalInput").ap()
        for c in range(NCHUNK)
    ]
    binary = nc.dram_tensor("binary", [nseg, P, SEGCOLS * B_DIM], mybir.dt.float32,
                            kind="ExternalInput").ap()
    gidx = nc.dram_tensor("gidx", [nseg, P, IDX_COLS], mybir.dt.int16,
                          kind="ExternalInput").ap()
    sidx = nc.dram_tensor("sidx", [nseg, P, IDX_COLS], mybir.dt.int16,
                          kind="ExternalInput").ap()
    counts = nc.dram_tensor("counts", [1, nseg * 2 * NCHUNK], mybir.dt.int32,
                            kind="ExternalInput").ap()
    out = nc.dram_tensor("out", [ne_pad, OUT_DIM], mybir.dt.float32,
                         kind="ExternalOutput").ap()
    out_bin = out.rearrange("(s n p) c -> s p n c", p=P, n=SEGCOLS)

    reg = nc.alloc_register(mybir.EngineType.Pool, "cnt")

    with tile.TileContext(nc) as tc, ExitStack() as ctx:
        cpool = ctx.enter_context(tc.tile_pool(name="c", bufs=1))
        gpool = ctx.enter_context(tc.tile_pool(name="g", bufs=2))
        ipool = ctx.enter_context(tc.tile_pool(name="i", bufs=2))
        bpool = ctx.enter_context(tc.tile_pool(name="b", bufs=2))

        cnt_sb = cpool.tile([1, nseg * 2 * NCHUNK], mybir.dt.int32, tag="cnt")
        nc.sync.dma_start(cnt_sb[:], counts[:, :])

        for s in range(nseg):
            gi = ipool.tile([P, IDX_COLS], mybir.dt.int16, tag="gi")
            si = ipool.tile([P, IDX_COLS], mybir.dt.int16, tag="si")
            nc.sync.dma_start(gi[:], gidx[s, :, :])
            nc.sync.dma_start(si[:], sidx[s, :, :])
            bt = bpool.tile([P, SEGCOLS * B_DIM], mybir.dt.float32, tag="bt")
            nc.sync.dma_start(bt[:], binary[s, :, :])

            for t in range(2):
                gt = gpool.tile([P, TCOLS * U_DIM], mybir.dt.float32, tag=f"gt{t}")
                gv = gt[:].rearrange("p (n e) -> p n e", e=U_DIM)
                for c in range(NCHUNK):
                    k = (s * 2 + t) * NCHUNK + c
                    icol = t * sum(CAPI) + sum(CAPI[:c])
                    scol = sum(CAPCOLS[:c])
                    nc.gpsimd.reg_load(reg, cnt_sb[0:1, k:k + 1])
                    nc.gpsimd.dma_gather(
                        out_ap=gv[:, scol:scol + CAPCOLS[c], :],
                        in_ap=chunks[c][:, :],
                        idxs_ap=gi[:, icol:icol + CAPI[c]],
                        num_idxs=CAPS[c],
                        num_idxs_reg=reg,
                        elem_size=U_DIM,
                        single_packet=False,
                    )
                    nc.gpsimd.dma_scatter_add(
                        out_ap=out[s * SEG:(s + 1) * SEG, t * U_DIM:(t + 1) * U_DIM],
                        in_ap=gv[:, scol:scol + CAPCOLS[c], :],
                        idxs_ap=si[:, icol:icol + CAPI[c]],
                        num_idxs=CAPS[c],
                        num_idxs_reg=reg,
                        elem_size=U_DIM,
                        elem_step=OUT_DIM,
                        single_packet=False,
                    )
            nc.sync.dma_start(out_bin[s, :, :, 2 * U_DIM:OUT_DIM],
                              bt[:].rearrange("p (n c) -> p n c", c=B_DIM))
    nc.compile()
    return nc


_NC_CACHE: dict = {}


def _get_nc(nseg: int):
    if nseg not in _NC_CACHE:
        _NC_CACHE[nseg] = _build_nc(nseg)
    return _NC_CACHE[nseg]


def _prepare(unary, binary, index1, index2):
    unary = np.ascontiguousarray(np.asarray(unary, dtype=np.float32))
    binary = np.ascontiguousarray(np.asarray(binary, dtype=np.float32))
    index1 = np.asarray(index1).astype(np.int32).ravel()
    index2 = np.asarray(index2).astype(np.int32).ravel()

    ne_total = binary.shape[0]
    per_core = -(-ne_total // N_CORES)
    nseg = -(-per_core // SEG)
    ne_pad = nseg * SEG
    nc = _get_nc(nseg)

    uchunks = {
        f"uc{c}": np.ascontiguousarray(unary[c * CHUNK:c * CHUNK + CHUNK_SIZES[c]])
        for c in range(NCHUNK)
    }

    in_maps = []
    counts_list = []
    for core in range(N_CORES):
        lo = core * per_core
        hi = min(lo + per_core, ne_total)
        n = hi - lo
        counts_list.append(n)

        b_pad = np.zeros((ne_pad, B_DIM), np.float32)
        b_pad[:n] = binary[lo:hi]
        bin_prep = np.ascontiguousarray(
            b_pad.reshape(nseg, SEGCOLS, P, B_DIM).transpose(0, 2, 1, 3)
            .reshape(nseg, P, SEGCOLS * B_DIM))

        gidx = np.full((nseg, P, IDX_COLS), -1, np.int16)
        sidx = np.full((nseg, P, IDX_COLS), -1, np.int16)
        cnts = np.zeros((1, nseg * 2 * NCHUNK), np.int32)
        for s in range(nseg):
            slo, shi = s * SEG, min((s + 1) * SEG, n)
            if slo >= shi:
                raise RuntimeError("empty segment (input too small/unbalanced)")
            for t, idx in enumerate((index1, index2)):
                seg_idx = idx[lo + slo:lo + shi]
                ch = seg_idx >> 15
                order = np.argsort(ch, kind="stable")
                cc = np.bincount(ch, minlength=NCHUNK)
                rel = (seg_idx[order] & 32767).astype(np.int16)
                pos = order.astype(np.int16)
                off = 0
                for c in range(NCHUNK):
                    k = (s * 2 + t) * NCHUNK + c
                    if not (0 < cc[c] <= CAPS[c]):
                        raise RuntimeError(f"count {cc[c]} outside (0, {CAPS[c]}]")
                    cnts[0, k] = cc[c]
                    icol = t * sum(CAPI) + sum(CAPI[:c])
                    gidx[s, :, icol:icol + CAPI[c]] = _wrap16(
                        rel[off:off + cc[c]], CAPI[c])
                    sidx[s, :, icol:icol + CAPI[c]] = _wrap16(
                        pos[off:off + cc[c]], CAPI[c])
                    off += cc[c]

        in_maps.append({
            **uchunks,
            "binary": bin_prep,
            "gidx": gidx,
            "sidx": sidx,
            "counts": cnts,
        })
    return nc, in_maps, counts_list, ne_total


def _assemble(res, counts_list, ne_total):
    out = np.empty((ne_total, OUT_DIM), dtype=np.float32)
    row = 0
    for core in range(N_CORES):
        out[row:row + counts_list[core]] = res.results[core]["out"][:counts_list[core]]
        row += counts_list[core]
    return out


def kernel(unary, binary, index1, index2):
    nc, in_maps, counts_list, ne_total = _prepare(unary, binary, index1, index2)
    res = run_bass_kernel_spmd(nc, in_maps, core_ids=list(range(N_CORES)))
    return _assemble(res, counts_list, ne_total)


# revision 5
# speedup vs baseline: 2.1181x; 2.1181x over previous
"""Trainium2 Bass kernel for nn_Join: out = concat(unary[idx1], unary[idx2], binary).

Strategy (data-parallel over edges, 8 cores):
  - 1M edges sharded 125000/core, padded to a multiple of 128.
  - unary table (51.2MB fp32) replicated per core; gathers are local
    HW indirect DMAs. The HW DGE supports exactly one index per SBUF
    partition per call, so rows are tiled p-outer: row = p*ncols + t.
    Gather block t uses offset column idx_sb[:, t] and lands 128 rows
    (512B each) in the out tile's column block t.
  - A supertile of S blocks shares one binary load and one large
    contiguous store.
  - With row = p*ncols + t, all DRAM APs are plain reshapes of the
    natural row order: no host-side permutation of any tensor.

Perf notes (measured on HW):
  - Each gpsimd indirect_dma_start occupies the Pool engine ~1.10us
    (SWDGE launch + 128-descriptor generation) + ~0.29us issue gap;
    2x977 launches/core -> ~2.7ms serialized Pool time. That IS the
    kernel's critical path; all data movement (320MB/core, ~0.9ms at
    358GB/s) hides under it.
  - Alternatives measured and rejected:
      * dma_gather/dma_scatter_add (batched custom ops): ucode costs
        ~7.6ns per STATIC num_idxs entry; gather+scatter for 500k rows
        -> 4.9ms Pool time. Slower.
      * multi-offset indirect DMA ([128,k] offset AP): HW uses only
        offset[p,0] and streams contiguously - not a k-row gather.
      * vector-indirect InstDMACopy on HWDGE queues (sync/scalar):
        device fault - RTL DGE cannot do indexed descriptors.
    The indexed-row descriptor rate (~9-11ns/row through one Pool pipe)
    is the hardware floor for this access pattern.
"""

import numpy as np
from contextlib import ExitStack

import concourse.bass as bass
import concourse.bacc as bacc
import concourse.tile as tile
import concourse.mybir as mybir
from concourse.bass_utils import run_bass_kernel_spmd

N_CORES = 8
U_NODES, U_DIM = 100000, 128
B_DIM = 64
OUT_DIM = 2 * U_DIM + B_DIM  # 320
P = 128
SUPER = 16  # gather blocks (columns) per supertile
OUT_BUFS = 4
B_BUFS = 3


def _build_nc(ncols: int, super_s: int = None, out_bufs: int = None,
              b_bufs: int = None):
    super_s = SUPER if super_s is None else super_s
    out_bufs = OUT_BUFS if out_bufs is None else out_bufs
    b_bufs = B_BUFS if b_bufs is None else b_bufs
    ne_pad = ncols * P
    nc = bacc.Bacc(
        "TRN2",
        target_bir_lowering=False,
        debug=False,
        enable_asserts=False,
        num_devices=N_CORES,
    )
    unary = nc.dram_tensor(
        "unary", [U_NODES, U_DIM], mybir.dt.float32, kind="ExternalInput"
    ).ap()
    binary = nc.dram_tensor(
        "binary", [ne_pad, B_DIM], mybir.dt.float32, kind="ExternalInput"
    ).ap()
    idx1 = nc.dram_tensor("idx1", [P, ncols], mybir.dt.int32, kind="ExternalInput").ap()
    idx2 = nc.dram_tensor("idx2", [P, ncols], mybir.dt.int32, kind="ExternalInput").ap()
    out = nc.dram_tensor(
        "out", [ne_pad, OUT_DIM], mybir.dt.float32, kind="ExternalOutput"
    ).ap()

    bin_v = binary.rearrange("(p n) c -> p n c", p=P)  # [128, ncols, 64]
    out_v = out.rearrange("(p n) c -> p n c", p=P)  # [128, ncols, 320]

    with tile.TileContext(nc) as tc, ExitStack() as ctx:
        idx_pool = ctx.enter_context(tc.tile_pool(name="idx", bufs=1))
        ot_pool = ctx.enter_context(tc.tile_pool(name="ot", bufs=out_bufs))
        bt_pool = ctx.enter_context(tc.tile_pool(name="bt", bufs=b_bufs))

        idx1_sb = idx_pool.tile([P, ncols], mybir.dt.int32, tag="idx1")
        idx2_sb = idx_pool.tile([P, ncols], mybir.dt.int32, tag="idx2")
        nc.sync.dma_start(idx1_sb[:], idx1[:, :])
        nc.sync.dma_start(idx2_sb[:], idx2[:, :])

        c0 = 0
        while c0 < ncols:
            S = min(super_s, ncols - c0)
            ot = ot_pool.tile([P, S * OUT_DIM], mybir.dt.float32, tag="ot")
            ov = ot[:].rearrange("p (s c) -> p s c", c=OUT_DIM)
            for s in range(S):
                nc.gpsimd.indirect_dma_start(
                    out=ov[:, s, 0:U_DIM],
                    out_offset=None,
                    in_=unary[:, :],
                    in_offset=bass.IndirectOffsetOnAxis(
                        ap=idx1_sb[:, c0 + s : c0 + s + 1], axis=0
                    ),
                )
                nc.gpsimd.indirect_dma_start(
                    out=ov[:, s, U_DIM : 2 * U_DIM],
                    out_offset=None,
                    in_=unary[:, :],
                    in_offset=bass.IndirectOffsetOnAxis(
                        ap=idx2_sb[:, c0 + s : c0 + s + 1], axis=0
                    ),
                )
            nc.sync.dma_start(ov[:, :, 2 * U_DIM : OUT_DIM], bin_v[:, c0 : c0 + S, :])
            nc.sync.dma_start(out_v[:, c0 : c0 + S, :], ot[:])
            c0 += S

    nc.compile()
    return nc


_NC_CACHE: dict = {}


def _get_nc(ncols: int):
    if ncols not in _NC_CACHE:
        _NC_CACHE[ncols] = _build_nc(ncols)
    return _NC_CACHE[ncols]


def _prepare(unary, binary, index1, index2):
    unary = np.ascontiguousarray(np.asarray(unary, dtype=np.float32))
    binary = np.ascontiguousarray(np.asarray(binary, dtype=np.float32))
    index1 = np.asarray(index1).astype(np.int32).ravel()
    index2 = np.asarray(index2).astype(np.int32).ravel()

    ne_total = binary.shape[0]
    per_core = -(-ne_total // N_CORES)
    ncols = -(-per_core // P)
    ne_pad = ncols * P
    nc = _get_nc(ncols)

    in_maps = []
    counts = []
    for c in range(N_CORES):
        lo = c * per_core
        hi = min(lo + per_core, ne_total)
        n = hi - lo
        counts.append(n)
        b = np.zeros((ne_pad, B_DIM), dtype=np.float32)
        b[:n] = binary[lo:hi]
        i1 = np.zeros(ne_pad, dtype=np.int32)
        i1[:n] = index1[lo:hi]
        i2 = np.zeros(ne_pad, dtype=np.int32)
        i2[:n] = index2[lo:hi]
        in_maps.append(
            {
                "unary": unary,
                "binary": b,
                "idx1": np.ascontiguousarray(i1.reshape(P, ncols)),
                "idx2": np.ascontiguousarray(i2.reshape(P, ncols)),
            }
        )
    return nc, in_maps, counts, ne_total


def _assemble(res, counts, ne_total):
    out = np.empty((ne_total, OUT_DIM), dtype=np.float32)
    row = 0
    for c in range(N_CORES):
        out[row : row + counts[c]] = res.results[c]["out"][: counts[c]]
        row += counts[c]
    return out


def kernel(unary, binary, index1, index2):
    nc, in_maps, counts, ne_total = _prepare(unary, binary, index1, index2)
    res = run_bass_kernel_spmd(nc, in_maps, core_ids=list(range(N_CORES)))
    return _assemble(res, counts, ne_total)
